# revision 7
# baseline (speedup 1.0000x reference)
"""Trainium2 Bass kernel for nn_Attention (llama-style attention block, GQA, RoPE).

v2: bf16 operand pipeline.

Distribution (8 NeuronCores, Megatron-style tensor parallel over heads):
  - Each core gets 4 Q heads + its matching 1 KV head (wq/wk/wv output-dim
    sharded). QKV projection keeps q/k/v in SBUF (no DRAM roundtrip).
  - Attention in transposed dataflow (head_dim on partitions, tokens on the
    free dim). Softmax denominators accumulate on DVE (e_acc) with a single
    [1,512] PE matmul per (head, q-block); sums ride the AllToAll.
  - Two per-batch AllToAlls reshard token-parallel; normalization after the
    AllToAll is DVE-only (partition_broadcast of the reciprocal row).
  - wo runs in a flipped dataflow: normalized attention (a_sb) is the
    stationary operand, wo.T streams as the moving operand, so the output
    lands as [token, dim] and the second AllToAll hides under the first
    batch's wo passes.
"""

import sys

if "/opt/trn_rl_repo" not in sys.path:
    sys.path.insert(0, "/opt/trn_rl_repo")

import numpy as np
import ml_dtypes

BF16 = ml_dtypes.bfloat16

N_CORES = 8
B, S, D = 2, 2048, 4096
N_HEADS = 32
N_KV_HEADS = 8
HEAD_DIM = 128
H_PER_CORE = N_HEADS // N_CORES          # 4 q heads per core
TOK = B * S                              # 4096 flattened tokens
QKV_M = H_PER_CORE * HEAD_DIM + 2 * HEAD_DIM  # 768 projection rows per core
PROJ_TOK = 512                           # token block in the projection stage
SQ_BLK = 512                             # sq block in attention
A2A_TOK = SQ_BLK // 2                    # tokens per rank per per-batch AllToAll
N_SQ_BLK = S // SQ_BLK                   # 4 per batch
N_TCHUNK = S // HEAD_DIM                 # 16 key chunks per batch
SCALE = 1.0 / float(np.sqrt(HEAD_DIM))
NKC = D // 128                           # 32 contraction chunks
N_ME = 8                                 # wo output-dim eighths
ME = D // N_ME                           # 512

# partition permutation for RoPE: pair (even, odd) lives 16 partitions apart
# inside a 32-partition quadrant, so the rotation is a single stream_shuffle.
_P = np.arange(128)
_I_OF_P = 16 * (_P // 32) + (_P % 32) % 16          # rope pair index 0..63
_IS_ODD = (_P % 32) >= 16
PERM = (2 * _I_OF_P + _IS_ODD.astype(np.int64)).astype(np.int64)  # orig row in head block
SHUF_MASK = [(i + 16) % 32 for i in range(32)]

_PROGRAMS = {}


def _build_program(mask_mode):
    """Build + compile the SPMD program. mask_mode in {'causal', 'none', 'general'}."""
    import concourse.bass as bass
    import concourse.mybir as mybir
    import concourse.tile as tile
    from concourse import bacc
    from concourse.masks import make_identity

    f32 = mybir.dt.float32
    bf16 = mybir.dt.bfloat16
    Exp = mybir.ActivationFunctionType.Exp

    nc = bacc.Bacc("TRN2", target_bir_lowering=False, debug=False,
                   num_devices=N_CORES)

    xT = nc.dram_tensor("xT", [D, TOK], bf16, kind="ExternalInput")
    wqkvT = nc.dram_tensor("wqkvT", [D, QKV_M], bf16, kind="ExternalInput")
    woT = nc.dram_tensor("woT", [D, D], bf16, kind="ExternalInput")
    # one-hot selector: selq[c, h*128+p] = (c == h); broadcasts reciprocal
    # row h across 128 partitions via a K=4 matmul
    selq = nc.dram_tensor("selq", [H_PER_CORE, H_PER_CORE * 128], bf16,
                          kind="ExternalInput")
    cos2 = nc.dram_tensor("cos2", [128, S], f32, kind="ExternalInput")
    sin2 = nc.dram_tensor("sin2", [128, S], f32, kind="ExternalInput")
    if mask_mode == "general":
        # additive mask stored transposed: maskT[k_pos, q_pos]
        maskT = nc.dram_tensor("maskT", [S, S], f32, kind="ExternalInput")
    # out[owned_token, dim]: rows 0:256 = batch0 tokens, 256:512 = batch1
    out_d = nc.dram_tensor("out", [2 * A2A_TOK, D], bf16, kind="ExternalOutput")

    xT_t = xT.ap().rearrange("(k p) t -> p k t", p=128)        # [128, 32, TOK]
    wqkvT_t = wqkvT.ap().rearrange("(k p) m -> p k m", p=128)  # [128, 32, 768]
    woT_t = woT.ap().rearrange("(k p) m -> p k m", p=128)      # [128, 32, 4096]

    with tile.TileContext(nc) as tc:
        with tc.tile_pool(name="const", bufs=1) as const, \
             tc.tile_pool(name="persist", bufs=1) as persist, \
             tc.tile_pool(name="dram", bufs=1, space="DRAM") as dram:
            a2a_in = [dram.tile([N_CORES, SQ_BLK, A2A_TOK], bf16,
                                name=f"a2a_in{b_}") for b_ in range(B)]
            a2a_out = [dram.tile([N_CORES, SQ_BLK, A2A_TOK], bf16,
                                 name=f"a2a_out{b_}") for b_ in range(B)]

            # persistent SBUF: projection outputs live here until attention
            q_sb = persist.tile([128, H_PER_CORE, TOK], bf16)   # 4MB
            k_sb = persist.tile([128, B, S], bf16)              # 1MB
            v_sb = persist.tile([128, B, S], bf16)              # 1MB
            v_nat = persist.tile([128, B, N_TCHUNK, 128], bf16)  # 2MB
            sel_sb = persist.tile([H_PER_CORE, H_PER_CORE * 128], bf16)
            nc.sync.dma_start(sel_sb[:], selq.ap())

            ones_col = const.tile([128, 1], bf16)     # lhsT for column sums
            ones_row = const.tile([1, 128], bf16)     # lhsT for row broadcast
            nc.vector.memset(ones_row[:], 1.0)
            nc.vector.memset(ones_col[:], 1.0)
            ident = const.tile([128, 128], bf16)      # bf16 PE transpose identity
            make_identity(nc, ident[:])
            if mask_mode == "causal":
                # strict lower-triangle additive mask: -1e9 where f < p.
                # A diagonal chunk at shift s only needs this on its
                # 128-wide strip [128s, 128s+128); columns below 128s are
                # fully masked (not computed), the rest fully visible.
                cmask = const.tile([128, 128], f32)
                nc.gpsimd.memset(cmask[:], 0.0)
                nc.gpsimd.affine_select(
                    out=cmask[:], in_=cmask[:],
                    pattern=[[1, 128]], base=0,
                    channel_multiplier=-1,
                    compare_op=mybir.AluOpType.is_ge, fill=-1e9,
                )

            # ---------------- stage 1: fused QKV projection + RoPE ----------------
            n_blk = TOK // PROJ_TOK
            HK = NKC // 2
            with tc.tile_pool(name="pj_w", bufs=1) as pj_w, \
                 tc.tile_pool(name="pj_x", bufs=2) as pj_x, \
                 tc.tile_pool(name="pj_cs", bufs=2) as pj_cs, \
                 tc.tile_pool(name="pj_t", bufs=2) as pj_t, \
                 tc.tile_pool(name="pj_ps", bufs=8, space="PSUM") as pj_ps:
                w_sb = pj_w.tile([128, NKC, QKV_M], bf16)
                x0 = slice(0, PROJ_TOK)
                xA0 = pj_x.tile([128, HK, PROJ_TOK], bf16, tag="xA")
                xB0 = pj_x.tile([128, HK, PROJ_TOK], bf16, tag="xB")
                # split the first x / weight loads so the k=0 pieces land
                # quickly and the first matmuls start early
                nc.sync.dma_start(xA0[:, 0:4, :], xT_t[:, 0:4, x0])
                nc.sync.dma_start(w_sb[:, 0:4, :], wqkvT_t[:, 0:4, :])
                nc.sync.dma_start(xA0[:, 4:HK, :], xT_t[:, 4:HK, x0])
                for kw in range(4, NKC, 4):
                    nc.sync.dma_start(w_sb[:, kw:kw + 4, :],
                                      wqkvT_t[:, kw:kw + 4, :])
                    if kw == 4:
                        nc.sync.dma_start(xB0[:], xT_t[:, HK:NKC, x0])
                for n in range(n_blk):
                    s0 = (n * PROJ_TOK) % S  # position within the batch
                    bn = n // (S // PROJ_TOK)  # batch of this token block
                    cols = slice(n * PROJ_TOK, (n + 1) * PROJ_TOK)
                    if n == 0:
                        xA, xB = xA0, xB0
                    else:
                        xA = pj_x.tile([128, HK, PROJ_TOK], bf16, tag="xA")
                        xB = pj_x.tile([128, HK, PROJ_TOK], bf16, tag="xB")
                        nc.sync.dma_start(xA[:], xT_t[:, 0:HK, cols])
                        nc.sync.dma_start(xB[:], xT_t[:, HK:NKC, cols])
                    c_sb = pj_cs.tile([128, PROJ_TOK], f32, tag="c")
                    s_sb = pj_cs.tile([128, PROJ_TOK], f32, tag="s")
                    nc.sync.dma_start(c_sb[:], cos2.ap()[:, s0:s0 + PROJ_TOK])
                    nc.sync.dma_start(s_sb[:], sin2.ap()[:, s0:s0 + PROJ_TOK])
                    pss = [pj_ps.tile([128, PROJ_TOK], f32, tag="ps",
                                      name=f"ps_{n}_{mi}")
                           for mi in range(QKV_M // 128)]
                    for k in range(NKC):
                        xsb = xA if k < HK else xB
                        xi = k if k < HK else k - HK
                        for m in range(QKV_M // 128):
                            nc.tensor.matmul(
                                pss[m][:], w_sb[:, k, m * 128:(m + 1) * 128],
                                xsb[:, xi, :],
                                start=(k == 0), stop=(k == NKC - 1))
                    for m in range(QKV_M // 128):  # q0..q3, k, v
                        ps = pss[m]
                        if m < 4:
                            dst = q_sb[:, m, cols]
                        elif m == 4:
                            dst = k_sb[:, bn, s0:s0 + PROJ_TOK]
                        else:
                            dst = v_sb[:, bn, s0:s0 + PROJ_TOK]
                        if m < 5:  # rope for q heads + k
                            tmp = pj_t.tile([128, PROJ_TOK], f32, tag="tmp")
                            rot = pj_t.tile([128, PROJ_TOK], f32, tag="rot")
                            t1 = pj_t.tile([128, PROJ_TOK], f32, tag="t1")
                            nc.scalar.copy(tmp[:], ps[:])
                            nc.vector.stream_shuffle(rot[:], tmp[:], SHUF_MASK)
                            nc.vector.tensor_mul(t1[:], tmp[:], c_sb[:])
                            nc.vector.tensor_mul(rot[:], rot[:], s_sb[:])
                            nc.vector.tensor_add(dst, t1[:], rot[:])
                        else:
                            nc.scalar.copy(dst, ps[:])

            # -------- stages 2+3: attention, AllToAll, norm, wo projection --------
            with tc.tile_pool(name="wo_w", bufs=2) as wo_w, \
                 tc.tile_pool(name="asb", bufs=1) as asb, \
                 tc.tile_pool(name="wo_o", bufs=2) as wo_o:
                a_sb = asb.tile([128, NKC, 2 * A2A_TOK], bf16)
                # prefetch the first wo moving block during attention
                wq0 = wo_w.tile([128, NKC, ME], bf16, tag="woq")
                nc.sync.dma_start(wq0[:], woT_t[:, :, 0:ME])

                with tc.tile_pool(name="at_e", bufs=4) as at_e, \
                     tc.tile_pool(name="at_acc", bufs=2) as at_acc, \
                     tc.tile_pool(name="at_bf", bufs=5) as at_bf, \
                     tc.tile_pool(name="at_o", bufs=5) as at_o, \
                     tc.tile_pool(name="nrm2", bufs=2) as nrm2, \
                     tc.tile_pool(name="ps_s", bufs=2, space="PSUM") as ps_s, \
                     tc.tile_pool(name="ps_av", bufs=5, space="PSUM") as ps_av, \
                     tc.tile_pool(name="ps_sm", bufs=1, space="PSUM") as ps_sm:
                    # DMA transposes serialize against collectives on the
                    # shared rings -- emit them all before the first AllToAll
                    for b in range(B):
                        for i in range(N_TCHUNK):
                            nc.sync.dma_start_transpose(
                                v_nat[:, b, i, :],
                                v_sb[:, b, i * 128:(i + 1) * 128])
                    def stage_a_sb(b):
                        # stage the (already normalized) AllToAll output
                        # into SBUF with plain DMAs; emission point chosen
                        # so these collective-gated DMAs never sit ahead of
                        # attention DMAs in the in-order DMA queues
                        for r in range(N_CORES):
                            nc.sync.dma_start(
                                a_sb[:, 4 * r:4 * r + 4,
                                     b * A2A_TOK:(b + 1) * A2A_TOK],
                                a2a_out[b][r, :, :]
                                .rearrange("(h p) t -> p h t", p=128))

                    for b in range(B):
                        pending_tail = None
                        for h in range(H_PER_CORE):
                            if b == 1 and h == H_PER_CORE - 1:
                                # a2a0 completed during b1's earlier heads
                                stage_a_sb(0)
                            # per-q-block reciprocals are taken straight off
                            # the sm PSUM (no SBUF-SBUF gather DMAs, which
                            # can queue behind collective-gated DMAs);
                            # normalization happens pre-AllToAll at end of h
                            av_tiles = []
                            recb_tiles = []
                            for j in range(N_SQ_BLK):
                                nchunk = (4 * j + 4 if mask_mode == "causal"
                                          else N_TCHUNK)
                                qs = q_sb[:, h,
                                          b * S + j * SQ_BLK:
                                          b * S + (j + 1) * SQ_BLK]
                                av = ps_av.tile([128, SQ_BLK], f32, tag="av")
                                accA = at_acc.tile([128, SQ_BLK], f32,
                                                   tag="eaccA")
                                accB = at_acc.tile([128, SQ_BLK], f32,
                                                   tag="eaccB")
                                sm = ps_sm.tile([1, SQ_BLK], f32, tag="sum")
                                for c in range(nchunk):
                                    # diagonal chunk at shift s: columns
                                    # below 128*s are fully masked -- skip
                                    # them; only the 128-wide strip at
                                    # [128s, 128s+128) needs the mask add
                                    if mask_mode == "causal" and c >= 4 * j:
                                        col0 = 128 * (c - 4 * j)
                                    else:
                                        col0 = 0
                                    w = SQ_BLK - col0
                                    cols = slice(col0, SQ_BLK)
                                    sp = ps_s.tile([128, SQ_BLK], f32,
                                                   tag="s")
                                    nc.tensor.matmul(
                                        sp[:, cols],
                                        k_sb[:, b, c * 128:(c + 1) * 128],
                                        qs[:, cols], start=True, stop=True)
                                    if mask_mode == "causal" and c >= 4 * j:
                                        nc.vector.tensor_add(
                                            sp[:, col0:col0 + 128],
                                            sp[:, col0:col0 + 128],
                                            cmask[:])
                                    elif mask_mode == "general":
                                        mt = at_e.tile([128, SQ_BLK], f32,
                                                       tag="mt")
                                        nc.sync.dma_start(
                                            mt[:],
                                            maskT.ap()[c * 128:(c + 1) * 128,
                                                       j * SQ_BLK:
                                                       (j + 1) * SQ_BLK])
                                        nc.vector.tensor_add(
                                            sp[:], sp[:], mt[:])
                                    e = at_e.tile([128, SQ_BLK], bf16,
                                                  tag="e")
                                    nc.scalar.activation(e[:, cols],
                                                         sp[:, cols], Exp,
                                                         scale=SCALE)
                                    nc.tensor.matmul(
                                        av[:, cols], v_nat[:, b, c, :],
                                        e[:, cols],
                                        start=(c == 0),
                                        stop=(c == nchunk - 1),
                                        skip_group_check=True)
                                    # two interleaved DVE accumulation
                                    # chains halve the serial add latency
                                    acc = accA if c % 2 == 0 else accB
                                    if c < 2:
                                        nc.vector.tensor_copy(
                                            acc[:, cols], e[:, cols])
                                    else:
                                        nc.vector.tensor_add(
                                            acc[:, cols], acc[:, cols],
                                            e[:, cols])
                                # emit the previous head's normalize tail
                                # here so its reciprocal chain hides under
                                # this head's first q-block matmuls
                                if j == 0 and pending_tail is not None:
                                    pending_tail()
                                    pending_tail = None
                                acc_bf = at_bf.tile([128, SQ_BLK], bf16,
                                                    tag="accbf")
                                # chainB's first chunk may start at col 128
                                # (j==0 causal); below that only chainA
                                # contributes
                                bcol0 = 128 if (mask_mode == "causal"
                                                and j == 0) else 0
                                if bcol0:
                                    nc.vector.tensor_copy(
                                        acc_bf[:, 0:bcol0], accA[:, 0:bcol0])
                                nc.vector.tensor_add(acc_bf[:, bcol0:],
                                                     accA[:, bcol0:],
                                                     accB[:, bcol0:])
                                nc.tensor.matmul(sm[:], ones_col[:],
                                                 acc_bf[:],
                                                 start=True, stop=True)
                                recj = at_acc.tile([1, SQ_BLK], f32,
                                                   tag="recj")
                                nc.vector.reciprocal_approx_fast(
                                    out=recj[:], in_=sm[:])
                                recjb = at_bf.tile([1, SQ_BLK], bf16,
                                                   tag="recjb")
                                nc.scalar.copy(recjb[:], recj[:])
                                recb_tiles.append(recjb)
                                av_tiles.append(av)

                            def make_tail(b=b, h=h, av_tiles=av_tiles,
                                          recb_tiles=recb_tiles):
                                def tail():
                                    for j in range(N_SQ_BLK):
                                        bc = ps_s.tile([128, SQ_BLK], f32,
                                                       tag="s")
                                        nc.tensor.matmul(
                                            bc[:], ones_row[:],
                                            recb_tiles[j][:],
                                            start=True, stop=True)
                                        bcs = at_bf.tile([128, SQ_BLK], bf16,
                                                         tag="bcs")
                                        nc.scalar.copy(bcs[:], bc[:])
                                        at = at_o.tile([128, SQ_BLK], bf16,
                                                       tag="at")
                                        nc.vector.tensor_mul(
                                            at[:], av_tiles[j][:], bcs[:])
                                        # tokens [512j, 512j+512) of batch b
                                        # span a2a blocks 2j and 2j+1
                                        nc.sync.dma_start(
                                            a2a_in[b][2 * j:2 * j + 2,
                                                      h * 128:(h + 1) * 128,
                                                      :]
                                            .rearrange("jb p t -> p jb t"),
                                            at[:].rearrange(
                                                "p (jb t) -> p jb t", jb=2))
                                return tail
                            pending_tail = make_tail()
                        pending_tail()
                        pending_tail = None
                        nc.gpsimd.collective_compute(
                            "AllToAll", mybir.AluOpType.bypass,
                            replica_groups=[list(range(N_CORES))],
                            ins=[a2a_in[b].opt()], outs=[a2a_out[b].opt()],
                        )

                with tc.tile_pool(name="wo_ps", bufs=4,
                                  space="PSUM") as wo_ps:
                    # batch-outer: all batch-0 passes run while AllToAll-1 is
                    # in flight (woT streams twice, which the DMA budget
                    # affords); batch-1 staging DMAs are emitted only after
                    # the last batch-0 pass so no collective-gated DMA can
                    # queue ahead of batch-0 work
                    for b in range(B):
                        if b == 1:
                            stage_a_sb(1)
                        for me in range(N_ME):
                            if b == 0 and me == 0:
                                wq_t = wq0
                            else:
                                wq_t = wo_w.tile([128, NKC, ME], bf16,
                                                 tag="woq")
                                nc.sync.dma_start(
                                    wq_t[:],
                                    woT_t[:, :, me * ME:(me + 1) * ME])
                            pst = [wo_ps.tile([128, ME], f32, tag="wops",
                                              name=f"wops_{me}_{b}_{tb}")
                                   for tb in range(2)]
                            for k in range(NKC):
                                for tb in range(2):
                                    c0 = b * A2A_TOK + tb * 128
                                    nc.tensor.matmul(
                                        pst[tb][:], a_sb[:, k, c0:c0 + 128],
                                        wq_t[:, k, :],
                                        start=(k == 0), stop=(k == NKC - 1))
                            for tb in range(2):
                                ob = wo_o.tile([128, ME], bf16, tag="ob")
                                nc.scalar.copy(ob[:], pst[tb][:])
                                r0 = b * A2A_TOK + tb * 128
                                nc.sync.dma_start(
                                    out_d.ap()[r0:r0 + 128,
                                               me * ME:(me + 1) * ME],
                                    ob[:])

    nc.compile()
    return nc


def _get_program(mask_mode):
    if mask_mode not in _PROGRAMS:
        _PROGRAMS[mask_mode] = _build_program(mask_mode)
    return _PROGRAMS[mask_mode]


def _classify_mask(m2):
    if not m2.any():
        return "none"
    causal_ref = np.triu(np.full((S, S), -1e9, dtype=np.float32), k=1)
    return "causal" if np.array_equal(m2, causal_ref) else "general"


def _prep_inputs(x, freqs_cos, freqs_sin, mask, wq, wk, wv, wo):
    """Host-side sharding / layout prep shared by kernel() and test.py."""
    m2 = np.asarray(mask, np.float32).reshape(S, S)
    mask_mode = _classify_mask(m2)

    xT = np.ascontiguousarray(
        np.asarray(x, np.float32).reshape(TOK, D).T.astype(BF16))
    woT = np.ascontiguousarray(np.asarray(wo, np.float32).T.astype(BF16))

    fc = np.asarray(freqs_cos, np.float32)
    fs = np.asarray(freqs_sin, np.float32)
    cos2 = np.ascontiguousarray(fc.T[_I_OF_P, :])            # [128, S]
    sgn = np.where(_IS_ODD, 1.0, -1.0).astype(np.float32)[:, None]
    sin2 = np.ascontiguousarray(fs.T[_I_OF_P, :] * sgn)

    def permute_heads(w):
        w4 = np.asarray(w, np.float32).reshape(-1, HEAD_DIM, D)
        return w4[:, PERM, :].reshape(-1, D)

    wq_p = permute_heads(wq)
    wk_p = permute_heads(wk)
    wv = np.asarray(wv, np.float32)

    selq = np.zeros((H_PER_CORE, H_PER_CORE * 128), dtype=BF16)
    for h in range(H_PER_CORE):
        selq[h, h * 128:(h + 1) * 128] = 1.0

    in_maps = []
    for c in range(N_CORES):
        wqkvT = np.ascontiguousarray(np.concatenate(
            [wq_p[c * 512:(c + 1) * 512], wk_p[c * 128:(c + 1) * 128],
             wv[c * 128:(c + 1) * 128]], axis=0).T.astype(BF16))  # [D, 768]
        m = {"xT": xT, "wqkvT": wqkvT, "woT": woT, "cos2": cos2,
             "sin2": sin2, "selq": selq}
        if mask_mode == "general":
            m["maskT"] = np.ascontiguousarray(m2.T)
        in_maps.append(m)
    return mask_mode, in_maps


def kernel(x, start_pos, freqs_cos, freqs_sin, mask, cache_k, cache_v,
           wq, wk, wv, wo):
    from concourse.bass_utils import run_bass_kernel_spmd

    assert int(start_pos) == 0, "kernel compiled for start_pos == 0"
    mask_mode, in_maps = _prep_inputs(x, freqs_cos, freqs_sin, mask,
                                      wq, wk, wv, wo)
    nc = _get_program(mask_mode)
    res = run_bass_kernel_spmd(nc, in_maps, list(range(N_CORES)))
    out = np.empty((B, S, D), dtype=np.float32)
    for c in range(N_CORES):
        blk = np.asarray(res.results[c]["out"], dtype=np.float32)  # [512, D]
        for b in range(B):
            out[b, A2A_TOK * c:A2A_TOK * (c + 1), :] = \
                blk[b * A2A_TOK:(b + 1) * A2A_TOK, :]
    return out


# revision 11
# speedup vs baseline: 1.0671x; 1.0671x over previous
"""Trainium2 Bass kernel for nn_Attention (llama-style attention block, GQA, RoPE).

v2: bf16 operand pipeline.

Distribution (8 NeuronCores, Megatron-style tensor parallel over heads):
  - Each core gets 4 Q heads + its matching 1 KV head (wq/wk/wv output-dim
    sharded). QKV projection keeps q/k/v in SBUF (no DRAM roundtrip).
  - Attention in transposed dataflow (head_dim on partitions, tokens on the
    free dim). Softmax denominators accumulate on DVE (e_acc) with a single
    [1,512] PE matmul per (head, q-block); sums ride the AllToAll.
  - Two per-batch AllToAlls reshard token-parallel; normalization after the
    AllToAll is DVE-only (partition_broadcast of the reciprocal row).
  - wo runs in a flipped dataflow: normalized attention (a_sb) is the
    stationary operand, wo.T streams as the moving operand, so the output
    lands as [token, dim] and the second AllToAll hides under the first
    batch's wo passes.
"""

import sys

if "/opt/trn_rl_repo" not in sys.path:
    sys.path.insert(0, "/opt/trn_rl_repo")

import numpy as np
import ml_dtypes

BF16 = ml_dtypes.bfloat16

N_CORES = 8
B, S, D = 2, 2048, 4096
N_HEADS = 32
N_KV_HEADS = 8
HEAD_DIM = 128
H_PER_CORE = N_HEADS // N_CORES          # 4 q heads per core
TOK = B * S                              # 4096 flattened tokens
QKV_M = H_PER_CORE * HEAD_DIM + 2 * HEAD_DIM  # 768 projection rows per core
PROJ_TOK = 512                           # token block in the projection stage
SQ_BLK = 512                             # sq block in attention
A2A_TOK = SQ_BLK // 2                    # tokens per rank per per-batch AllToAll
N_SQ_BLK = S // SQ_BLK                   # 4 per batch
N_TCHUNK = S // HEAD_DIM                 # 16 key chunks per batch
SCALE = 1.0 / float(np.sqrt(HEAD_DIM))
NKC = D // 128                           # 32 contraction chunks
N_ME = 8                                 # wo output-dim eighths
ME = D // N_ME                           # 512

# partition permutation for RoPE: pair (even, odd) lives 16 partitions apart
# inside a 32-partition quadrant, so the rotation is a single stream_shuffle.
_P = np.arange(128)
_I_OF_P = 16 * (_P // 32) + (_P % 32) % 16          # rope pair index 0..63
_IS_ODD = (_P % 32) >= 16
PERM = (2 * _I_OF_P + _IS_ODD.astype(np.int64)).astype(np.int64)  # orig row in head block
SHUF_MASK = [(i + 16) % 32 for i in range(32)]

_PROGRAMS = {}


def _build_program(mask_mode):
    """Build + compile the SPMD program. mask_mode in {'causal', 'none', 'general'}."""
    import concourse.bass as bass
    import concourse.mybir as mybir
    import concourse.tile as tile
    from concourse import bacc
    from concourse.masks import make_identity

    f32 = mybir.dt.float32
    bf16 = mybir.dt.bfloat16
    Exp = mybir.ActivationFunctionType.Exp

    nc = bacc.Bacc("TRN2", target_bir_lowering=False, debug=False,
                   num_devices=N_CORES)

    xT = nc.dram_tensor("xT", [D, TOK], bf16, kind="ExternalInput")
    wqkvT = nc.dram_tensor("wqkvT", [D, QKV_M], bf16, kind="ExternalInput")
    woT = nc.dram_tensor("woT", [D, D], bf16, kind="ExternalInput")
    # one-hot selector: selq[c, h*128+p] = (c == h); broadcasts reciprocal
    # row h across 128 partitions via a K=4 matmul
    selq = nc.dram_tensor("selq", [H_PER_CORE, H_PER_CORE * 128], bf16,
                          kind="ExternalInput")
    cos2 = nc.dram_tensor("cos2", [128, S], f32, kind="ExternalInput")
    sin2 = nc.dram_tensor("sin2", [128, S], f32, kind="ExternalInput")
    if mask_mode == "general":
        # additive mask stored transposed: maskT[k_pos, q_pos]
        maskT = nc.dram_tensor("maskT", [S, S], f32, kind="ExternalInput")
    # out[owned_token, dim]: rows 0:256 = batch0 tokens, 256:512 = batch1
    out_d = nc.dram_tensor("out", [2 * A2A_TOK, D], bf16, kind="ExternalOutput")

    xT_t = xT.ap().rearrange("(k p) t -> p k t", p=128)        # [128, 32, TOK]
    wqkvT_t = wqkvT.ap().rearrange("(k p) m -> p k m", p=128)  # [128, 32, 768]
    woT_t = woT.ap().rearrange("(k p) m -> p k m", p=128)      # [128, 32, 4096]

    with tile.TileContext(nc) as tc:
        with tc.tile_pool(name="const", bufs=1) as const, \
             tc.tile_pool(name="persist", bufs=1) as persist, \
             tc.tile_pool(name="dram", bufs=1, space="DRAM") as dram:
            a2a_in = [dram.tile([N_CORES, SQ_BLK, A2A_TOK], bf16,
                                name=f"a2a_in{b_}") for b_ in range(B)]
            a2a_out = [dram.tile([N_CORES, SQ_BLK, A2A_TOK], bf16,
                                 name=f"a2a_out{b_}") for b_ in range(B)]

            # persistent SBUF: projection outputs live here until attention
            q_sb = persist.tile([128, H_PER_CORE, TOK], bf16)   # 4MB
            k_sb = persist.tile([128, B, S], bf16)              # 1MB
            v_sb = persist.tile([128, B, S], bf16)              # 1MB
            v_nat = persist.tile([128, B, N_TCHUNK, 128], bf16)  # 2MB
            sel_sb = persist.tile([H_PER_CORE, H_PER_CORE * 128], bf16)
            nc.sync.dma_start(sel_sb[:], selq.ap())

            ones_col = const.tile([128, 1], bf16)     # lhsT for column sums
            ones_row = const.tile([1, 128], bf16)     # lhsT for row broadcast
            nc.vector.memset(ones_row[:], 1.0)
            nc.vector.memset(ones_col[:], 1.0)
            ident = const.tile([128, 128], bf16)      # bf16 PE transpose identity
            make_identity(nc, ident[:])
            if mask_mode == "causal":
                # strict lower-triangle additive mask: -1e9 where f < p.
                # A diagonal chunk at shift s only needs this on its
                # 128-wide strip [128s, 128s+128); columns below 128s are
                # fully masked (not computed), the rest fully visible.
                cmask = const.tile([128, 128], f32)
                nc.gpsimd.memset(cmask[:], 0.0)
                nc.gpsimd.affine_select(
                    out=cmask[:], in_=cmask[:],
                    pattern=[[1, 128]], base=0,
                    channel_multiplier=-1,
                    compare_op=mybir.AluOpType.is_ge, fill=-1e9,
                )

            # ---------------- stage 1: fused QKV projection + RoPE ----------------
            n_blk = TOK // PROJ_TOK
            HK = NKC // 2
            with tc.tile_pool(name="pj_w", bufs=1) as pj_w, \
                 tc.tile_pool(name="pj_x", bufs=2) as pj_x, \
                 tc.tile_pool(name="pj_cs", bufs=2) as pj_cs, \
                 tc.tile_pool(name="pj_t", bufs=2) as pj_t, \
                 tc.tile_pool(name="pj_ps", bufs=8, space="PSUM") as pj_ps:
                w_sb = pj_w.tile([128, NKC, QKV_M], bf16)
                x0 = slice(0, PROJ_TOK)
                xA0 = pj_x.tile([128, HK, PROJ_TOK], bf16, tag="xA")
                xB0 = pj_x.tile([128, HK, PROJ_TOK], bf16, tag="xB")
                # split the first x / weight loads so the k=0 pieces land
                # quickly and the first matmuls start early
                nc.sync.dma_start(xA0[:, 0:4, :], xT_t[:, 0:4, x0])
                nc.sync.dma_start(w_sb[:, 0:4, :], wqkvT_t[:, 0:4, :])
                nc.sync.dma_start(xA0[:, 4:HK, :], xT_t[:, 4:HK, x0])
                for kw in range(4, NKC, 4):
                    nc.sync.dma_start(w_sb[:, kw:kw + 4, :],
                                      wqkvT_t[:, kw:kw + 4, :])
                    if kw == 4:
                        nc.sync.dma_start(xB0[:], xT_t[:, HK:NKC, x0])
                for n in range(n_blk):
                    s0 = (n * PROJ_TOK) % S  # position within the batch
                    bn = n // (S // PROJ_TOK)  # batch of this token block
                    cols = slice(n * PROJ_TOK, (n + 1) * PROJ_TOK)
                    if n == 0:
                        xA, xB = xA0, xB0
                    else:
                        xA = pj_x.tile([128, HK, PROJ_TOK], bf16, tag="xA")
                        xB = pj_x.tile([128, HK, PROJ_TOK], bf16, tag="xB")
                        nc.sync.dma_start(xA[:], xT_t[:, 0:HK, cols])
                        nc.sync.dma_start(xB[:], xT_t[:, HK:NKC, cols])
                    c_sb = pj_cs.tile([128, PROJ_TOK], f32, tag="c")
                    s_sb = pj_cs.tile([128, PROJ_TOK], f32, tag="s")
                    nc.sync.dma_start(c_sb[:], cos2.ap()[:, s0:s0 + PROJ_TOK])
                    nc.sync.dma_start(s_sb[:], sin2.ap()[:, s0:s0 + PROJ_TOK])
                    pss = [pj_ps.tile([128, PROJ_TOK], f32, tag="ps",
                                      name=f"ps_{n}_{mi}")
                           for mi in range(QKV_M // 128)]
                    for k in range(NKC):
                        xsb = xA if k < HK else xB
                        xi = k if k < HK else k - HK
                        for m in range(QKV_M // 128):
                            nc.tensor.matmul(
                                pss[m][:], w_sb[:, k, m * 128:(m + 1) * 128],
                                xsb[:, xi, :],
                                start=(k == 0), stop=(k == NKC - 1))
                    for m in range(QKV_M // 128):  # q0..q3, k, v
                        ps = pss[m]
                        if m < 4:
                            dst = q_sb[:, m, cols]
                        elif m == 4:
                            dst = k_sb[:, bn, s0:s0 + PROJ_TOK]
                        else:
                            dst = v_sb[:, bn, s0:s0 + PROJ_TOK]
                        if m < 5:  # rope for q heads + k
                            tmp = pj_t.tile([128, PROJ_TOK], f32, tag="tmp")
                            rot = pj_t.tile([128, PROJ_TOK], f32, tag="rot")
                            t1 = pj_t.tile([128, PROJ_TOK], f32, tag="t1")
                            nc.scalar.copy(tmp[:], ps[:])
                            nc.vector.stream_shuffle(rot[:], tmp[:], SHUF_MASK)
                            nc.vector.tensor_mul(t1[:], tmp[:], c_sb[:])
                            nc.vector.tensor_mul(rot[:], rot[:], s_sb[:])
                            nc.vector.tensor_add(dst, t1[:], rot[:])
                        else:
                            nc.scalar.copy(dst, ps[:])

            # -------- stages 2+3: attention, AllToAll, norm, wo projection --------
            with tc.tile_pool(name="wo_w", bufs=2) as wo_w, \
                 tc.tile_pool(name="asb", bufs=1) as asb, \
                 tc.tile_pool(name="wo_o", bufs=2) as wo_o:
                a_sb = asb.tile([128, NKC, 2 * A2A_TOK], bf16)
                # prefetch the first wo moving block during attention
                wq0 = wo_w.tile([128, NKC, ME], bf16, tag="woq")
                nc.sync.dma_start(wq0[:], woT_t[:, :, 0:ME])

                with tc.tile_pool(name="at_e", bufs=4) as at_e, \
                     tc.tile_pool(name="at_acc", bufs=2) as at_acc, \
                     tc.tile_pool(name="at_bf", bufs=5) as at_bf, \
                     tc.tile_pool(name="at_o", bufs=5) as at_o, \
                     tc.tile_pool(name="nrm2", bufs=2) as nrm2, \
                     tc.tile_pool(name="ps_s", bufs=2, space="PSUM") as ps_s, \
                     tc.tile_pool(name="ps_av", bufs=5, space="PSUM") as ps_av, \
                     tc.tile_pool(name="ps_sm", bufs=1, space="PSUM") as ps_sm:
                    # DMA transposes serialize against collectives on the
                    # shared rings -- emit them all before the first AllToAll
                    for b in range(B):
                        for i in range(N_TCHUNK):
                            nc.sync.dma_start_transpose(
                                v_nat[:, b, i, :],
                                v_sb[:, b, i * 128:(i + 1) * 128])
                    def stage_a_sb(b):
                        # stage the (already normalized) AllToAll output
                        # into SBUF with plain DMAs; emission point chosen
                        # so these collective-gated DMAs never sit ahead of
                        # attention DMAs in the in-order DMA queues
                        for r in range(N_CORES):
                            nc.sync.dma_start(
                                a_sb[:, 4 * r:4 * r + 4,
                                     b * A2A_TOK:(b + 1) * A2A_TOK],
                                a2a_out[b][r, :, :]
                                .rearrange("(h p) t -> p h t", p=128))

                    for b in range(B):
                        pending_tail = None
                        for h in range(H_PER_CORE):
                            # per-q-block reciprocals are taken straight off
                            # the sm PSUM (no SBUF-SBUF gather DMAs, which
                            # can queue behind collective-gated DMAs);
                            # normalization happens pre-AllToAll at end of h
                            av_tiles = []
                            recb_tiles = []
                            for j in range(N_SQ_BLK):
                                nchunk = (4 * j + 4 if mask_mode == "causal"
                                          else N_TCHUNK)
                                qs = q_sb[:, h,
                                          b * S + j * SQ_BLK:
                                          b * S + (j + 1) * SQ_BLK]
                                av = ps_av.tile([128, SQ_BLK], f32, tag="av")
                                accA = at_acc.tile([128, SQ_BLK], f32,
                                                   tag="eaccA")
                                accB = at_acc.tile([128, SQ_BLK], f32,
                                                   tag="eaccB")
                                sm = ps_sm.tile([1, SQ_BLK], f32, tag="sum")
                                for c in range(nchunk):
                                    # diagonal chunk at shift s: columns
                                    # below 128*s are fully masked -- skip
                                    # them; only the 128-wide strip at
                                    # [128s, 128s+128) needs the mask add
                                    if mask_mode == "causal" and c >= 4 * j:
                                        col0 = 128 * (c - 4 * j)
                                    else:
                                        col0 = 0
                                    w = SQ_BLK - col0
                                    cols = slice(col0, SQ_BLK)
                                    sp = ps_s.tile([128, SQ_BLK], f32,
                                                   tag="s")
                                    nc.tensor.matmul(
                                        sp[:, cols],
                                        k_sb[:, b, c * 128:(c + 1) * 128],
                                        qs[:, cols], start=True, stop=True)
                                    if mask_mode == "causal" and c >= 4 * j:
                                        nc.vector.tensor_add(
                                            sp[:, col0:col0 + 128],
                                            sp[:, col0:col0 + 128],
                                            cmask[:])
                                    elif mask_mode == "general":
                                        mt = at_e.tile([128, SQ_BLK], f32,
                                                       tag="mt")
                                        nc.sync.dma_start(
                                            mt[:],
                                            maskT.ap()[c * 128:(c + 1) * 128,
                                                       j * SQ_BLK:
                                                       (j + 1) * SQ_BLK])
                                        nc.vector.tensor_add(
                                            sp[:], sp[:], mt[:])
                                    e = at_e.tile([128, SQ_BLK], bf16,
                                                  tag="e")
                                    nc.scalar.activation(e[:, cols],
                                                         sp[:, cols], Exp,
                                                         scale=SCALE)
                                    nc.tensor.matmul(
                                        av[:, cols], v_nat[:, b, c, :],
                                        e[:, cols],
                                        start=(c == 0),
                                        stop=(c == nchunk - 1),
                                        skip_group_check=True)
                                    # two interleaved DVE accumulation
                                    # chains halve the serial add latency
                                    acc = accA if c % 2 == 0 else accB
                                    if c < 2:
                                        nc.vector.tensor_copy(
                                            acc[:, cols], e[:, cols])
                                    else:
                                        nc.vector.tensor_add(
                                            acc[:, cols], acc[:, cols],
                                            e[:, cols])
                                # emit the previous head's normalize tail
                                # here so its reciprocal chain hides under
                                # this head's first q-block matmuls
                                if j == 0 and pending_tail is not None:
                                    pending_tail()
                                    pending_tail = None
                                acc_bf = at_bf.tile([128, SQ_BLK], bf16,
                                                    tag="accbf")
                                # chainB's first chunk may start at col 128
                                # (j==0 causal); below that only chainA
                                # contributes
                                bcol0 = 128 if (mask_mode == "causal"
                                                and j == 0) else 0
                                if bcol0:
                                    nc.vector.tensor_copy(
                                        acc_bf[:, 0:bcol0], accA[:, 0:bcol0])
                                nc.vector.tensor_add(acc_bf[:, bcol0:],
                                                     accA[:, bcol0:],
                                                     accB[:, bcol0:])
                                nc.tensor.matmul(sm[:], ones_col[:],
                                                 acc_bf[:],
                                                 start=True, stop=True)
                                recj = at_acc.tile([1, SQ_BLK], f32,
                                                   tag="recj")
                                nc.vector.reciprocal_approx_fast(
                                    out=recj[:], in_=sm[:])
                                recjb = at_bf.tile([1, SQ_BLK], bf16,
                                                   tag="recjb")
                                nc.scalar.copy(recjb[:], recj[:])
                                recb_tiles.append(recjb)
                                av_tiles.append(av)

                            def make_tail(b=b, h=h, av_tiles=av_tiles,
                                          recb_tiles=recb_tiles):
                                def tail():
                                    for j in range(N_SQ_BLK):
                                        bc = ps_s.tile([128, SQ_BLK], f32,
                                                       tag="s")
                                        nc.tensor.matmul(
                                            bc[:], ones_row[:],
                                            recb_tiles[j][:],
                                            start=True, stop=True)
                                        bcs = at_bf.tile([128, SQ_BLK], bf16,
                                                         tag="bcs")
                                        nc.scalar.copy(bcs[:], bc[:])
                                        at = at_o.tile([128, SQ_BLK], bf16,
                                                       tag="at")
                                        nc.vector.tensor_mul(
                                            at[:], av_tiles[j][:], bcs[:])
                                        # tokens [512j, 512j+512) of batch b
                                        # span a2a blocks 2j and 2j+1
                                        nc.sync.dma_start(
                                            a2a_in[b][2 * j:2 * j + 2,
                                                      h * 128:(h + 1) * 128,
                                                      :]
                                            .rearrange("jb p t -> p jb t"),
                                            at[:].rearrange(
                                                "p (jb t) -> p jb t", jb=2))
                                return tail
                            pending_tail = make_tail()
                        pending_tail()
                        pending_tail = None
                        nc.gpsimd.collective_compute(
                            "AllToAll", mybir.AluOpType.bypass,
                            replica_groups=[list(range(N_CORES))],
                            ins=[a2a_in[b].opt()], outs=[a2a_out[b].opt()],
                        )
                        # safe to stage immediately: attention no longer has
                        # compute-feeding DMAs that could queue behind these
                        # collective-gated transfers
                        stage_a_sb(b)

                with tc.tile_pool(name="wo_ps", bufs=4,
                                  space="PSUM") as wo_ps:
                    # prefetch the me=1 weight block too (plain DRAM read)
                    wq1 = wo_w.tile([128, NKC, ME], bf16, tag="woq")
                    nc.sync.dma_start(wq1[:], woT_t[:, :, ME:2 * ME])
                    # first two b0 passes run before any a2a1-gated pass so
                    # the second collective's tail latency is covered
                    pass_order = [(0, 0), (1, 0), (0, 1), (1, 1)]
                    pass_order += [(me, b) for me in range(2, N_ME)
                                   for b in range(B)]
                    wq_tiles = {0: wq0, 1: wq1}
                    for me, b in pass_order:
                        if me not in wq_tiles:
                            wq_tiles[me] = wo_w.tile([128, NKC, ME], bf16,
                                                     tag="woq",
                                                     name=f"wq_{me}")
                            nc.sync.dma_start(
                                wq_tiles[me][:],
                                woT_t[:, :, me * ME:(me + 1) * ME])
                        wq_t = wq_tiles[me]
                        pst = [wo_ps.tile([128, ME], f32, tag="wops",
                                          name=f"wops_{me}_{b}_{tb}")
                               for tb in range(2)]
                        for k in range(NKC):
                            for tb in range(2):
                                c0 = b * A2A_TOK + tb * 128
                                nc.tensor.matmul(
                                    pst[tb][:], a_sb[:, k, c0:c0 + 128],
                                    wq_t[:, k, :],
                                    start=(k == 0), stop=(k == NKC - 1))
                        for tb in range(2):
                            ob = wo_o.tile([128, ME], bf16, tag="ob")
                            nc.scalar.copy(ob[:], pst[tb][:])
                            r0 = b * A2A_TOK + tb * 128
                            nc.sync.dma_start(
                                out_d.ap()[r0:r0 + 128,
                                           me * ME:(me + 1) * ME],
                                ob[:])

    nc.compile()
    return nc


def _get_program(mask_mode):
    if mask_mode not in _PROGRAMS:
        _PROGRAMS[mask_mode] = _build_program(mask_mode)
    return _PROGRAMS[mask_mode]


def _classify_mask(m2):
    if not m2.any():
        return "none"
    causal_ref = np.triu(np.full((S, S), -1e9, dtype=np.float32), k=1)
    return "causal" if np.array_equal(m2, causal_ref) else "general"


def _prep_inputs(x, freqs_cos, freqs_sin, mask, wq, wk, wv, wo):
    """Host-side sharding / layout prep shared by kernel() and test.py."""
    m2 = np.asarray(mask, np.float32).reshape(S, S)
    mask_mode = _classify_mask(m2)

    xT = np.ascontiguousarray(
        np.asarray(x, np.float32).reshape(TOK, D).T.astype(BF16))
    woT = np.ascontiguousarray(np.asarray(wo, np.float32).T.astype(BF16))

    fc = np.asarray(freqs_cos, np.float32)
    fs = np.asarray(freqs_sin, np.float32)
    cos2 = np.ascontiguousarray(fc.T[_I_OF_P, :])            # [128, S]
    sgn = np.where(_IS_ODD, 1.0, -1.0).astype(np.float32)[:, None]
    sin2 = np.ascontiguousarray(fs.T[_I_OF_P, :] * sgn)

    def permute_heads(w):
        w4 = np.asarray(w, np.float32).reshape(-1, HEAD_DIM, D)
        return w4[:, PERM, :].reshape(-1, D)

    wq_p = permute_heads(wq)
    wk_p = permute_heads(wk)
    wv = np.asarray(wv, np.float32)

    selq = np.zeros((H_PER_CORE, H_PER_CORE * 128), dtype=BF16)
    for h in range(H_PER_CORE):
        selq[h, h * 128:(h + 1) * 128] = 1.0

    in_maps = []
    for c in range(N_CORES):
        wqkvT = np.ascontiguousarray(np.concatenate(
            [wq_p[c * 512:(c + 1) * 512], wk_p[c * 128:(c + 1) * 128],
             wv[c * 128:(c + 1) * 128]], axis=0).T.astype(BF16))  # [D, 768]
        m = {"xT": xT, "wqkvT": wqkvT, "woT": woT, "cos2": cos2,
             "sin2": sin2, "selq": selq}
        if mask_mode == "general":
            m["maskT"] = np.ascontiguousarray(m2.T)
        in_maps.append(m)
    return mask_mode, in_maps


def kernel(x, start_pos, freqs_cos, freqs_sin, mask, cache_k, cache_v,
           wq, wk, wv, wo):
    from concourse.bass_utils import run_bass_kernel_spmd

    assert int(start_pos) == 0, "kernel compiled for start_pos == 0"
    mask_mode, in_maps = _prep_inputs(x, freqs_cos, freqs_sin, mask,
                                      wq, wk, wv, wo)
    nc = _get_program(mask_mode)
    res = run_bass_kernel_spmd(nc, in_maps, list(range(N_CORES)))
    out = np.empty((B, S, D), dtype=np.float32)
    for c in range(N_CORES):
        blk = np.asarray(res.results[c]["out"], dtype=np.float32)  # [512, D]
        for b in range(B):
            out[b, A2A_TOK * c:A2A_TOK * (c + 1), :] = \
                blk[b * A2A_TOK:(b + 1) * A2A_TOK, :]
    return out


# revision 12
# speedup vs baseline: 1.0719x; 1.0045x over previous
"""Trainium2 Bass kernel for nn_Attention (llama-style attention block, GQA, RoPE).

v2: bf16 operand pipeline.

Distribution (8 NeuronCores, Megatron-style tensor parallel over heads):
  - Each core gets 4 Q heads + its matching 1 KV head (wq/wk/wv output-dim
    sharded). QKV projection keeps q/k/v in SBUF (no DRAM roundtrip).
  - Attention in transposed dataflow (head_dim on partitions, tokens on the
    free dim). Softmax denominators accumulate on DVE (e_acc) with a single
    [1,512] PE matmul per (head, q-block); sums ride the AllToAll.
  - Two per-batch AllToAlls reshard token-parallel; normalization after the
    AllToAll is DVE-only (partition_broadcast of the reciprocal row).
  - wo runs in a flipped dataflow: normalized attention (a_sb) is the
    stationary operand, wo.T streams as the moving operand, so the output
    lands as [token, dim] and the second AllToAll hides under the first
    batch's wo passes.
"""

import sys

if "/opt/trn_rl_repo" not in sys.path:
    sys.path.insert(0, "/opt/trn_rl_repo")

import numpy as np
import ml_dtypes

BF16 = ml_dtypes.bfloat16

N_CORES = 8
B, S, D = 2, 2048, 4096
N_HEADS = 32
N_KV_HEADS = 8
HEAD_DIM = 128
H_PER_CORE = N_HEADS // N_CORES          # 4 q heads per core
TOK = B * S                              # 4096 flattened tokens
QKV_M = H_PER_CORE * HEAD_DIM + 2 * HEAD_DIM  # 768 projection rows per core
PROJ_TOK = 512                           # token block in the projection stage
SQ_BLK = 512                             # sq block in attention
A2A_TOK = SQ_BLK // 2                    # tokens per rank per per-batch AllToAll
N_SQ_BLK = S // SQ_BLK                   # 4 per batch
N_TCHUNK = S // HEAD_DIM                 # 16 key chunks per batch
SCALE = 1.0 / float(np.sqrt(HEAD_DIM))
NKC = D // 128                           # 32 contraction chunks
N_ME = 8                                 # wo output-dim eighths
ME = D // N_ME                           # 512

# partition permutation for RoPE: pair (even, odd) lives 16 partitions apart
# inside a 32-partition quadrant, so the rotation is a single stream_shuffle.
_P = np.arange(128)
_I_OF_P = 16 * (_P // 32) + (_P % 32) % 16          # rope pair index 0..63
_IS_ODD = (_P % 32) >= 16
PERM = (2 * _I_OF_P + _IS_ODD.astype(np.int64)).astype(np.int64)  # orig row in head block
SHUF_MASK = [(i + 16) % 32 for i in range(32)]

_PROGRAMS = {}


def _build_program(mask_mode):
    """Build + compile the SPMD program. mask_mode in {'causal', 'none', 'general'}."""
    import concourse.bass as bass
    import concourse.mybir as mybir
    import concourse.tile as tile
    from concourse import bacc
    from concourse.masks import make_identity

    f32 = mybir.dt.float32
    bf16 = mybir.dt.bfloat16
    Exp = mybir.ActivationFunctionType.Exp

    nc = bacc.Bacc("TRN2", target_bir_lowering=False, debug=False,
                   num_devices=N_CORES)

    xT = nc.dram_tensor("xT", [D, TOK], bf16, kind="ExternalInput")
    wqkvT = nc.dram_tensor("wqkvT", [D, QKV_M], bf16, kind="ExternalInput")
    woT = nc.dram_tensor("woT", [D, D], bf16, kind="ExternalInput")
    # one-hot selector: selq[c, h*128+p] = (c == h); broadcasts reciprocal
    # row h across 128 partitions via a K=4 matmul
    selq = nc.dram_tensor("selq", [H_PER_CORE, H_PER_CORE * 128], bf16,
                          kind="ExternalInput")
    cos2 = nc.dram_tensor("cos2", [128, S], f32, kind="ExternalInput")
    sin2 = nc.dram_tensor("sin2", [128, S], f32, kind="ExternalInput")
    if mask_mode == "general":
        # additive mask stored transposed: maskT[k_pos, q_pos]
        maskT = nc.dram_tensor("maskT", [S, S], f32, kind="ExternalInput")
    # out[owned_token, dim]: rows 0:256 = batch0 tokens, 256:512 = batch1
    out_d = nc.dram_tensor("out", [2 * A2A_TOK, D], bf16, kind="ExternalOutput")

    xT_t = xT.ap().rearrange("(k p) t -> p k t", p=128)        # [128, 32, TOK]
    wqkvT_t = wqkvT.ap().rearrange("(k p) m -> p k m", p=128)  # [128, 32, 768]
    woT_t = woT.ap().rearrange("(k p) m -> p k m", p=128)      # [128, 32, 4096]

    with tile.TileContext(nc) as tc:
        with tc.tile_pool(name="const", bufs=1) as const, \
             tc.tile_pool(name="persist", bufs=1) as persist, \
             tc.tile_pool(name="dram", bufs=1, space="DRAM") as dram:
            a2a_in = [dram.tile([N_CORES, SQ_BLK, A2A_TOK], bf16,
                                name=f"a2a_in{b_}") for b_ in range(B)]
            a2a_out = [dram.tile([N_CORES, SQ_BLK, A2A_TOK], bf16,
                                 name=f"a2a_out{b_}") for b_ in range(B)]

            # persistent SBUF: projection outputs live here until attention
            q_sb = persist.tile([128, H_PER_CORE, TOK], bf16)   # 4MB
            k_sb = persist.tile([128, B, S], bf16)              # 1MB
            v_sb = persist.tile([128, B, S], bf16)              # 1MB
            v_nat = persist.tile([128, B, N_TCHUNK, 128], bf16)  # 2MB
            sel_sb = persist.tile([H_PER_CORE, H_PER_CORE * 128], bf16)
            nc.sync.dma_start(sel_sb[:], selq.ap())

            ones_col = const.tile([128, 1], bf16)     # lhsT for column sums
            ones_row = const.tile([1, 128], bf16)     # lhsT for row broadcast
            nc.vector.memset(ones_row[:], 1.0)
            nc.vector.memset(ones_col[:], 1.0)
            ident = const.tile([128, 128], bf16)      # bf16 PE transpose identity
            make_identity(nc, ident[:])
            if mask_mode == "causal":
                # strict lower-triangle additive mask: -1e9 where f < p.
                # A diagonal chunk at shift s only needs this on its
                # 128-wide strip [128s, 128s+128); columns below 128s are
                # fully masked (not computed), the rest fully visible.
                cmask = const.tile([128, 128], f32)
                nc.gpsimd.memset(cmask[:], 0.0)
                nc.gpsimd.affine_select(
                    out=cmask[:], in_=cmask[:],
                    pattern=[[1, 128]], base=0,
                    channel_multiplier=-1,
                    compare_op=mybir.AluOpType.is_ge, fill=-1e9,
                )

            # ---------------- stage 1: fused QKV projection + RoPE ----------------
            n_blk = TOK // PROJ_TOK
            HK = NKC // 2
            with tc.tile_pool(name="pj_w", bufs=1) as pj_w, \
                 tc.tile_pool(name="pj_x", bufs=2) as pj_x, \
                 tc.tile_pool(name="pj_cs", bufs=2) as pj_cs, \
                 tc.tile_pool(name="pj_t", bufs=2) as pj_t, \
                 tc.tile_pool(name="pj_ps", bufs=8, space="PSUM") as pj_ps:
                w_sb = pj_w.tile([128, NKC, QKV_M], bf16)
                x0 = slice(0, PROJ_TOK)
                xA0 = pj_x.tile([128, HK, PROJ_TOK], bf16, tag="xA")
                xB0 = pj_x.tile([128, HK, PROJ_TOK], bf16, tag="xB")
                # split the first x / weight loads so the k=0 pieces land
                # quickly and the first matmuls start early
                nc.sync.dma_start(xA0[:, 0:2, :], xT_t[:, 0:2, x0])
                nc.sync.dma_start(w_sb[:, 0:2, :], wqkvT_t[:, 0:2, :])
                nc.sync.dma_start(xA0[:, 2:4, :], xT_t[:, 2:4, x0])
                nc.sync.dma_start(w_sb[:, 2:4, :], wqkvT_t[:, 2:4, :])
                nc.sync.dma_start(xA0[:, 4:HK, :], xT_t[:, 4:HK, x0])
                for kw in range(4, NKC, 4):
                    nc.sync.dma_start(w_sb[:, kw:kw + 4, :],
                                      wqkvT_t[:, kw:kw + 4, :])
                    if kw == 4:
                        nc.sync.dma_start(xB0[:], xT_t[:, HK:NKC, x0])
                for n in range(n_blk):
                    s0 = (n * PROJ_TOK) % S  # position within the batch
                    bn = n // (S // PROJ_TOK)  # batch of this token block
                    cols = slice(n * PROJ_TOK, (n + 1) * PROJ_TOK)
                    if n == 0:
                        xA, xB = xA0, xB0
                    else:
                        xA = pj_x.tile([128, HK, PROJ_TOK], bf16, tag="xA")
                        xB = pj_x.tile([128, HK, PROJ_TOK], bf16, tag="xB")
                        nc.sync.dma_start(xA[:], xT_t[:, 0:HK, cols])
                        nc.sync.dma_start(xB[:], xT_t[:, HK:NKC, cols])
                    c_sb = pj_cs.tile([128, PROJ_TOK], f32, tag="c")
                    s_sb = pj_cs.tile([128, PROJ_TOK], f32, tag="s")
                    nc.sync.dma_start(c_sb[:], cos2.ap()[:, s0:s0 + PROJ_TOK])
                    nc.sync.dma_start(s_sb[:], sin2.ap()[:, s0:s0 + PROJ_TOK])
                    pss = [pj_ps.tile([128, PROJ_TOK], f32, tag="ps",
                                      name=f"ps_{n}_{mi}")
                           for mi in range(QKV_M // 128)]
                    for k in range(NKC):
                        xsb = xA if k < HK else xB
                        xi = k if k < HK else k - HK
                        for m in range(QKV_M // 128):
                            nc.tensor.matmul(
                                pss[m][:], w_sb[:, k, m * 128:(m + 1) * 128],
                                xsb[:, xi, :],
                                start=(k == 0), stop=(k == NKC - 1))
                    for m in range(QKV_M // 128):  # q0..q3, k, v
                        ps = pss[m]
                        if m < 4:
                            dst = q_sb[:, m, cols]
                        elif m == 4:
                            dst = k_sb[:, bn, s0:s0 + PROJ_TOK]
                        else:
                            dst = v_sb[:, bn, s0:s0 + PROJ_TOK]
                        if m < 5:  # rope for q heads + k
                            tmp = pj_t.tile([128, PROJ_TOK], f32, tag="tmp")
                            rot = pj_t.tile([128, PROJ_TOK], f32, tag="rot")
                            t1 = pj_t.tile([128, PROJ_TOK], f32, tag="t1")
                            nc.scalar.copy(tmp[:], ps[:])
                            nc.vector.stream_shuffle(rot[:], tmp[:], SHUF_MASK)
                            nc.vector.tensor_mul(t1[:], tmp[:], c_sb[:])
                            nc.vector.tensor_mul(rot[:], rot[:], s_sb[:])
                            nc.vector.tensor_add(dst, t1[:], rot[:])
                        else:
                            nc.scalar.copy(dst, ps[:])

            # -------- stages 2+3: attention, AllToAll, norm, wo projection --------
            with tc.tile_pool(name="wo_w", bufs=2) as wo_w, \
                 tc.tile_pool(name="asb", bufs=1) as asb, \
                 tc.tile_pool(name="wo_o", bufs=3) as wo_o:
                a_sb = asb.tile([128, NKC, 2 * A2A_TOK], bf16)
                # prefetch the first wo moving block during attention
                wq0 = wo_w.tile([128, NKC, ME], bf16, tag="woq")
                nc.sync.dma_start(wq0[:], woT_t[:, :, 0:ME])

                with tc.tile_pool(name="at_e", bufs=5) as at_e, \
                     tc.tile_pool(name="at_acc", bufs=2) as at_acc, \
                     tc.tile_pool(name="at_bf", bufs=5) as at_bf, \
                     tc.tile_pool(name="at_o", bufs=5) as at_o, \
                     tc.tile_pool(name="nrm2", bufs=2) as nrm2, \
                     tc.tile_pool(name="ps_s", bufs=2, space="PSUM") as ps_s, \
                     tc.tile_pool(name="ps_av", bufs=5, space="PSUM") as ps_av, \
                     tc.tile_pool(name="ps_sm", bufs=1, space="PSUM") as ps_sm:
                    # DMA transposes serialize against collectives on the
                    # shared rings -- emit them all before the first AllToAll
                    for b in range(B):
                        for i in range(N_TCHUNK):
                            nc.sync.dma_start_transpose(
                                v_nat[:, b, i, :],
                                v_sb[:, b, i * 128:(i + 1) * 128])
                    def stage_a_sb(b):
                        # stage the (already normalized) AllToAll output
                        # into SBUF with plain DMAs; emission point chosen
                        # so these collective-gated DMAs never sit ahead of
                        # attention DMAs in the in-order DMA queues
                        for r in range(N_CORES):
                            nc.sync.dma_start(
                                a_sb[:, 4 * r:4 * r + 4,
                                     b * A2A_TOK:(b + 1) * A2A_TOK],
                                a2a_out[b][r, :, :]
                                .rearrange("(h p) t -> p h t", p=128))

                    for b in range(B):
                        pending_tail = None
                        for h in range(H_PER_CORE):
                            # per-q-block reciprocals are taken straight off
                            # the sm PSUM (no SBUF-SBUF gather DMAs, which
                            # can queue behind collective-gated DMAs);
                            # normalization happens pre-AllToAll at end of h
                            av_tiles = []
                            recb_tiles = []
                            for j in range(N_SQ_BLK):
                                nchunk = (4 * j + 4 if mask_mode == "causal"
                                          else N_TCHUNK)
                                qs = q_sb[:, h,
                                          b * S + j * SQ_BLK:
                                          b * S + (j + 1) * SQ_BLK]
                                av = ps_av.tile([128, SQ_BLK], f32, tag="av")
                                accA = at_acc.tile([128, SQ_BLK], f32,
                                                   tag="eaccA")
                                accB = at_acc.tile([128, SQ_BLK], f32,
                                                   tag="eaccB")
                                sm = ps_sm.tile([1, SQ_BLK], f32, tag="sum")
                                for c in range(nchunk):
                                    # diagonal chunk at shift s: columns
                                    # below 128*s are fully masked -- skip
                                    # them; only the 128-wide strip at
                                    # [128s, 128s+128) needs the mask add
                                    if mask_mode == "causal" and c >= 4 * j:
                                        col0 = 128 * (c - 4 * j)
                                    else:
                                        col0 = 0
                                    w = SQ_BLK - col0
                                    cols = slice(col0, SQ_BLK)
                                    sp = ps_s.tile([128, SQ_BLK], f32,
                                                   tag="s")
                                    nc.tensor.matmul(
                                        sp[:, cols],
                                        k_sb[:, b, c * 128:(c + 1) * 128],
                                        qs[:, cols], start=True, stop=True)
                                    if mask_mode == "causal" and c >= 4 * j:
                                        nc.vector.tensor_add(
                                            sp[:, col0:col0 + 128],
                                            sp[:, col0:col0 + 128],
                                            cmask[:])
                                    elif mask_mode == "general":
                                        mt = at_e.tile([128, SQ_BLK], f32,
                                                       tag="mt")
                                        nc.sync.dma_start(
                                            mt[:],
                                            maskT.ap()[c * 128:(c + 1) * 128,
                                                       j * SQ_BLK:
                                                       (j + 1) * SQ_BLK])
                                        nc.vector.tensor_add(
                                            sp[:], sp[:], mt[:])
                                    e = at_e.tile([128, SQ_BLK], bf16,
                                                  tag="e")
                                    nc.scalar.activation(e[:, cols],
                                                         sp[:, cols], Exp,
                                                         scale=SCALE)
                                    nc.tensor.matmul(
                                        av[:, cols], v_nat[:, b, c, :],
                                        e[:, cols],
                                        start=(c == 0),
                                        stop=(c == nchunk - 1),
                                        skip_group_check=True)
                                    # two interleaved DVE accumulation
                                    # chains halve the serial add latency
                                    acc = accA if c % 2 == 0 else accB
                                    if c < 2:
                                        nc.vector.tensor_copy(
                                            acc[:, cols], e[:, cols])
                                    else:
                                        nc.vector.tensor_add(
                                            acc[:, cols], acc[:, cols],
                                            e[:, cols])
                                # emit the previous head's normalize tail
                                # here so its reciprocal chain hides under
                                # this head's first q-block matmuls
                                if j == 0 and pending_tail is not None:
                                    pending_tail()
                                    pending_tail = None
                                acc_bf = at_bf.tile([128, SQ_BLK], bf16,
                                                    tag="accbf")
                                # chainB's first chunk may start at col 128
                                # (j==0 causal); below that only chainA
                                # contributes
                                bcol0 = 128 if (mask_mode == "causal"
                                                and j == 0) else 0
                                if bcol0:
                                    nc.vector.tensor_copy(
                                        acc_bf[:, 0:bcol0], accA[:, 0:bcol0])
                                nc.vector.tensor_add(acc_bf[:, bcol0:],
                                                     accA[:, bcol0:],
                                                     accB[:, bcol0:])
                                nc.tensor.matmul(sm[:], ones_col[:],
                                                 acc_bf[:],
                                                 start=True, stop=True)
                                recj = at_acc.tile([1, SQ_BLK], f32,
                                                   tag="recj")
                                nc.vector.reciprocal_approx_fast(
                                    out=recj[:], in_=sm[:])
                                recjb = at_bf.tile([1, SQ_BLK], bf16,
                                                   tag="recjb")
                                nc.scalar.copy(recjb[:], recj[:])
                                recb_tiles.append(recjb)
                                av_tiles.append(av)

                            def make_tail(b=b, h=h, av_tiles=av_tiles,
                                          recb_tiles=recb_tiles):
                                def tail():
                                    for j in range(N_SQ_BLK):
                                        bc = ps_s.tile([128, SQ_BLK], f32,
                                                       tag="s")
                                        nc.tensor.matmul(
                                            bc[:], ones_row[:],
                                            recb_tiles[j][:],
                                            start=True, stop=True)
                                        bcs = at_bf.tile([128, SQ_BLK], bf16,
                                                         tag="bcs")
                                        nc.scalar.copy(bcs[:], bc[:])
                                        at = at_o.tile([128, SQ_BLK], bf16,
                                                       tag="at")
                                        nc.vector.tensor_mul(
                                            at[:], av_tiles[j][:], bcs[:])
                                        # tokens [512j, 512j+512) of batch b
                                        # span a2a blocks 2j and 2j+1
                                        nc.sync.dma_start(
                                            a2a_in[b][2 * j:2 * j + 2,
                                                      h * 128:(h + 1) * 128,
                                                      :]
                                            .rearrange("jb p t -> p jb t"),
                                            at[:].rearrange(
                                                "p (jb t) -> p jb t", jb=2))
                                return tail
                            pending_tail = make_tail()
                        pending_tail()
                        pending_tail = None
                        nc.gpsimd.collective_compute(
                            "AllToAll", mybir.AluOpType.bypass,
                            replica_groups=[list(range(N_CORES))],
                            ins=[a2a_in[b].opt()], outs=[a2a_out[b].opt()],
                        )
                        # safe to stage immediately: attention no longer has
                        # compute-feeding DMAs that could queue behind these
                        # collective-gated transfers
                        stage_a_sb(b)

                with tc.tile_pool(name="wo_ps", bufs=4,
                                  space="PSUM") as wo_ps:
                    # prefetch the me=1 weight block too (plain DRAM read)
                    wq1 = wo_w.tile([128, NKC, ME], bf16, tag="woq")
                    nc.sync.dma_start(wq1[:], woT_t[:, :, ME:2 * ME])
                    # first two b0 passes run before any a2a1-gated pass so
                    # the second collective's tail latency is covered
                    pass_order = [(0, 0), (1, 0), (0, 1), (1, 1)]
                    pass_order += [(me, b) for me in range(2, N_ME)
                                   for b in range(B)]
                    wq_tiles = {0: wq0, 1: wq1}
                    for me, b in pass_order:
                        if me not in wq_tiles:
                            wq_tiles[me] = wo_w.tile([128, NKC, ME], bf16,
                                                     tag="woq",
                                                     name=f"wq_{me}")
                            nc.sync.dma_start(
                                wq_tiles[me][:],
                                woT_t[:, :, me * ME:(me + 1) * ME])
                        wq_t = wq_tiles[me]
                        pst = [wo_ps.tile([128, ME], f32, tag="wops",
                                          name=f"wops_{me}_{b}_{tb}")
                               for tb in range(2)]
                        for k in range(NKC):
                            for tb in range(2):
                                c0 = b * A2A_TOK + tb * 128
                                nc.tensor.matmul(
                                    pst[tb][:], a_sb[:, k, c0:c0 + 128],
                                    wq_t[:, k, :],
                                    start=(k == 0), stop=(k == NKC - 1))
                        for tb in range(2):
                            ob = wo_o.tile([128, ME], bf16, tag="ob")
                            nc.scalar.copy(ob[:], pst[tb][:])
                            r0 = b * A2A_TOK + tb * 128
                            nc.sync.dma_start(
                                out_d.ap()[r0:r0 + 128,
                                           me * ME:(me + 1) * ME],
                                ob[:])

    nc.compile()
    return nc


def _get_program(mask_mode):
    if mask_mode not in _PROGRAMS:
        _PROGRAMS[mask_mode] = _build_program(mask_mode)
    return _PROGRAMS[mask_mode]


def _classify_mask(m2):
    if not m2.any():
        return "none"
    causal_ref = np.triu(np.full((S, S), -1e9, dtype=np.float32), k=1)
    return "causal" if np.array_equal(m2, causal_ref) else "general"


def _prep_inputs(x, freqs_cos, freqs_sin, mask, wq, wk, wv, wo):
    """Host-side sharding / layout prep shared by kernel() and test.py."""
    m2 = np.asarray(mask, np.float32).reshape(S, S)
    mask_mode = _classify_mask(m2)

    xT = np.ascontiguousarray(
        np.asarray(x, np.float32).reshape(TOK, D).T.astype(BF16))
    woT = np.ascontiguousarray(np.asarray(wo, np.float32).T.astype(BF16))

    fc = np.asarray(freqs_cos, np.float32)
    fs = np.asarray(freqs_sin, np.float32)
    cos2 = np.ascontiguousarray(fc.T[_I_OF_P, :])            # [128, S]
    sgn = np.where(_IS_ODD, 1.0, -1.0).astype(np.float32)[:, None]
    sin2 = np.ascontiguousarray(fs.T[_I_OF_P, :] * sgn)

    def permute_heads(w):
        w4 = np.asarray(w, np.float32).reshape(-1, HEAD_DIM, D)
        return w4[:, PERM, :].reshape(-1, D)

    wq_p = permute_heads(wq)
    wk_p = permute_heads(wk)
    wv = np.asarray(wv, np.float32)

    selq = np.zeros((H_PER_CORE, H_PER_CORE * 128), dtype=BF16)
    for h in range(H_PER_CORE):
        selq[h, h * 128:(h + 1) * 128] = 1.0

    in_maps = []
    for c in range(N_CORES):
        wqkvT = np.ascontiguousarray(np.concatenate(
            [wq_p[c * 512:(c + 1) * 512], wk_p[c * 128:(c + 1) * 128],
             wv[c * 128:(c + 1) * 128]], axis=0).T.astype(BF16))  # [D, 768]
        m = {"xT": xT, "wqkvT": wqkvT, "woT": woT, "cos2": cos2,
             "sin2": sin2, "selq": selq}
        if mask_mode == "general":
            m["maskT"] = np.ascontiguousarray(m2.T)
        in_maps.append(m)
    return mask_mode, in_maps


def kernel(x, start_pos, freqs_cos, freqs_sin, mask, cache_k, cache_v,
           wq, wk, wv, wo):
    from concourse.bass_utils import run_bass_kernel_spmd

    assert int(start_pos) == 0, "kernel compiled for start_pos == 0"
    mask_mode, in_maps = _prep_inputs(x, freqs_cos, freqs_sin, mask,
                                      wq, wk, wv, wo)
    nc = _get_program(mask_mode)
    res = run_bass_kernel_spmd(nc, in_maps, list(range(N_CORES)))
    out = np.empty((B, S, D), dtype=np.float32)
    for c in range(N_CORES):
        blk = np.asarray(res.results[c]["out"], dtype=np.float32)  # [512, D]
        for b in range(B):
            out[b, A2A_TOK * c:A2A_TOK * (c + 1), :] = \
                blk[b * A2A_TOK:(b + 1) * A2A_TOK, :]
    return out
